# revision 19
# baseline (speedup 1.0000x reference)
"""Trainium2 Bass kernel for nn_CrossMed4 (CrossMed-style GRU-over-GRU model).

v4 strategy (8 NeuronCores, data-parallel over the patient batch B=16 -> 2/core):
- All four monitor embedding streams (lab/inj x item/value) are expanded on
  the PE from host-built fp8 one-hot matrices instead of SWDGE dma_gather
  (whose Q7 descriptor generation, ~8ns/row, was the machine bottleneck):
  stream[d, tok] = sum_vc tableT[vc, d]^T @ OH[vc, tok], accumulated over
  128-row vocab chunks in PSUM. fp8 one-hots are exact (0/1) and halve the
  HBM traffic; tables stay fp16.
- Token order t = g*24 + l (g = mi*32 + bv chunk-local group), so the pair
  product (DVE, fp16, one PSUM operand max) reduces over a contiguous
  innermost-24 axis straight into labT[d, g] -- no reduction matmuls, no
  transposes, no padding.
- Everything on the PE is 16-bit or fp8 (no fp32 hi/lo split, fast weight
  load); fp32 only in PSUM accumulation and the u-gate elementwise path.
- GRU state is fp16 end-to-end; r-gate stays fp32 for the u-path, z-gate is
  produced fp16 for the blend path. Gate biases are folded into ACT-engine
  Identity copies (per-partition bias); input projections accumulate onto
  ACT-preloaded PSUM. r/z and n gates live in SEPARATE PSUM banks (start=True
  marks the whole 2KB bank pending-zero, which would wipe the preload).
- Visit-level code features (cond/proc/drug) still use dma_gather (tiny:
  768 rows each) and reduce+transpose in one shot: six accumulating matmuls
  with the gathered rank as stationary and a 0/1 collapse matrix as moving.
"""
import numpy as np
import ml_dtypes

try:
    import concourse.bass as bass  # noqa: F401
except ImportError:
    import sys
    sys.path.insert(0, "/opt/trn_rl_repo")

import concourse.bacc as bacc
import concourse.bass as bass
import concourse.mybir as mybir
import concourse.tile as tile
from concourse.bass_utils import run_bass_kernel_spmd

F32 = mybir.dt.float32
F8 = mybir.dt.float8e4
BF16 = mybir.dt.float16
I16 = mybir.dt.int16
BF = np.float16
F8NP = ml_dtypes.float8_e4m3

B, V, M, L, D, OUT = 16, 16, 32, 24, 128, 193
VOCAB = {"cond": 5000, "proc": 2000, "drug": 600, "lab_item": 700,
         "lab_value": 200, "inj_item": 400, "inj_value": 200}
NCORES = 8
BL = B // NCORES            # 2 patients per core
NBV = BL * V                # 32 visit groups
TCH = 4                     # monitor steps per chunk
NCHUNK = M // TCH           # 8
GC = NBV * TCH              # 128 groups per chunk (= mi*32 + bv)
TOKC = GC * L               # 3072 tokens per chunk per stream
VIDX = NBV * L              # 768 visit-stream tokens (6 ranks)
AF = mybir.ActivationFunctionType

OHSPEC = (("lab_value", "oh_lab", "vt_lab", 2),
          ("inj_value", "oh_inj", "vt_inj", 2),
          ("inj_item", "oh_inji", "vt_inji", 4),
          ("lab_item", "oh_labi", "vt_labi", 6))


# --------------------------------------------------------------------------
# host-side packing
# --------------------------------------------------------------------------

def _wrap_idx(flat):
    # token i lives at [i % 16, i // 16]; the gather ucode's Q7 cores each
    # read their own 16-partition band, so replicate to all 8 bands.
    n = flat.shape[0]
    return np.tile(flat.reshape(n // 16, 16).T, (8, 1)).astype(np.int16)


def _packT(w_keys):  # [K, 3D, D] -> [128, K*3*128] fp16, col (k*3+gi)*128+d'
    k = w_keys.shape[0]
    out = np.zeros((128, k * 3 * 128), dtype=np.float32)
    for ki in range(k):
        for gi in range(3):
            out[:, (ki * 3 + gi) * 128:(ki * 3 + gi + 1) * 128] = \
                w_keys[ki, gi * 128:(gi + 1) * 128, :].T
    return out.astype(BF)


def _bias_cols(bih, bhh, keys):  # [128, len(keys)*3] f32, col k*3+gi
    cols = []
    for k in keys:
        for gi in range(3):
            b = bih[k][gi * D:(gi + 1) * D].copy()
            if gi < 2:
                b += bhh[k][gi * D:(gi + 1) * D]
            cols.append(b)
    return np.stack(cols, axis=1).astype(np.float32)


def _prep_shared(inputs):
    f = {k: np.asarray(v, dtype=np.float32) for k, v in inputs.items()
         if not k.startswith("tok_")}
    sh = {}
    for name in ("cond", "proc", "drug"):
        sh["emb_" + name] = f["emb_" + name].astype(BF)
    # tables packed for one-hot matmuls: vtp[p, c*128+d] = T[c*128+p, d]
    for name, _, vtag, nvc in OHSPEC:
        pad = np.zeros((nvc * 128, D), dtype=np.float32)
        pad[:VOCAB[name]] = f["emb_" + name]
        sh[vtag] = np.ascontiguousarray(
            pad.reshape(nvc, 128, D).transpose(1, 0, 2).reshape(128, nvc * 128)
        ).astype(BF)

    mwih, mwhh = f["mgru_wih"], f["mgru_whh"]
    mbih, mbhh = f["mgru_bih"], f["mgru_bhh"]
    vwih, vwhh = f["vgru_wih"], f["vgru_whh"]
    vbih, vbhh = f["vgru_bih"], f["vgru_bhh"]

    sh["mwhhT"] = _packT(mwhh)                   # [128, 1920]
    sh["mwihT012"] = _packT(mwih[0:3])           # [128, 1152]
    sh["mwihT34"] = _packT(mwih[3:5])            # [128, 768]
    sh["vwhhT"] = _packT(vwhh)                   # [128, 2688]
    sh["vwihT04"] = _packT(vwih[0:5])            # [128, 1920]
    sh["mb012c"] = _bias_cols(mbih, mbhh, [0, 1, 2])   # [128, 9]
    sh["mb34c"] = _bias_cols(mbih, mbhh, [3, 4])       # [128, 6]
    sh["vb04c"] = _bias_cols(vbih, vbhh, [0, 1, 2, 3, 4])  # [128, 15]
    sh["mbhn_bc"] = np.repeat(mbhh[:, 2 * D:].T, NBV, axis=1).astype(np.float32)
    sh["vbhn_bc"] = np.repeat(vbhh[:, 2 * D:].T, BL, axis=1).astype(np.float32)
    # keys 5,6 (weight/age): xg = u_k * wa[bv] + c_k  (rank-1)
    u_rows, c_cols = [], []
    for k in (5, 6):
        u_rows.append(vwih[k] @ f["info_w"][k - 5])
        cv = vwih[k] @ f["info_b"][k - 5] + vbih[k]
        cv[:2 * D] += vbhh[k][:2 * D]
        for gi in range(3):
            c_cols.append(cv[gi * D:(gi + 1) * D])
    sh["vxg56u"] = np.concatenate(u_rows)[None, :].astype(BF)   # [1, 768]
    sh["vb56c"] = np.stack(c_cols, axis=1).astype(np.float32)   # [128, 6]
    # collapse matrix for visit streams: S4[bv*4+cb, bv] = 1
    s4 = np.zeros((128, NBV), dtype=np.float32)
    for bv in range(NBV):
        s4[bv * 4:(bv + 1) * 4, bv] = 1.0
    sh["S4"] = s4.astype(BF)
    fcw = np.zeros((128, 7 * OUT), dtype=np.float32)
    for k in range(7):
        fcw[:, k * OUT:(k + 1) * OUT] = f["fc_w"][k * D:(k + 1) * D, :]
    sh["fcw"] = fcw.astype(BF)
    sh["fcb2"] = np.tile(f["fc_b"][None, :], (BL, 1)).astype(np.float32)
    return sh


def _mon_flat(tok):
    """tok [BL,V,M,L] int -> flat [NCHUNK*TOKC] with order
    flat[c*TOKC + (mi*NBV + b*V + v)*L + l] = tok[b, v, c*TCH+mi, l]."""
    t = np.asarray(tok).reshape(BL, V, NCHUNK, TCH, L)
    return np.ascontiguousarray(t.transpose(2, 3, 0, 1, 4)).reshape(-1)


def _prep_core(inputs, shared, core):
    b0 = core * BL
    m = dict(shared)
    # one-hot matrices [128, NCHUNK*nvc*TOKC] fp8 (0/1 exact)
    prow = np.arange(128)[:, None]
    for name, tag, _, nvc in OHSPEC:
        flat = _mon_flat(np.asarray(inputs["tok_" + name])[b0:b0 + BL])
        toks = flat.reshape(NCHUNK, TOKC)
        oh = np.zeros((128, NCHUNK, nvc, TOKC), dtype=F8NP)
        for c in range(NCHUNK):
            for vc in range(nvc):
                oh[:, c, vc, :] = (toks[c][None, :] == vc * 128 + prow)
        m[tag] = np.ascontiguousarray(oh).reshape(128, NCHUNK * nvc * TOKC)
    # visit-stream indices: flat[r*128 + (b*V+v)*4 + cb] = tok[b,v,cb*6+r]
    for name in ("cond", "proc", "drug"):
        t = np.asarray(inputs["tok_" + name])[b0:b0 + BL].reshape(BL, V, 4, 6)
        flat = np.ascontiguousarray(t.transpose(3, 0, 1, 2)).reshape(-1)
        m["idx_" + name] = _wrap_idx(flat)
    wa = np.zeros((1, 64), dtype=np.float32)
    wa[0, :NBV] = np.asarray(inputs["weight"], np.float32)[b0:b0 + BL].reshape(NBV)
    wa[0, NBV:] = np.asarray(inputs["age"], np.float32)[b0:b0 + BL].reshape(NBV)
    m["wa"] = wa.astype(BF)
    return m


# --------------------------------------------------------------------------
# device program
# --------------------------------------------------------------------------

CONSTS = (("mwhhT", [128, 1920], BF16), ("mwihT012", [128, 1152], BF16),
          ("mwihT34", [128, 768], BF16), ("vwhhT", [128, 2688], BF16),
          ("vwihT04", [128, 1920], BF16), ("mb012c", [128, 9], F32),
          ("mb34c", [128, 6], F32), ("vb04c", [128, 15], F32),
          ("mbhn_bc", [128, 160], F32), ("vbhn_bc", [128, 14], F32),
          ("vxg56u", [1, 768], BF16), ("vb56c", [128, 6], F32),
          ("S4", [128, NBV], BF16),
          ("fcw", [128, 7 * OUT], BF16), ("fcb2", [BL, OUT], F32),
          ("wa", [1, 64], BF16), ("vt_lab", [128, 256], BF16),
          ("vt_inj", [128, 256], BF16), ("vt_inji", [128, 512], BF16),
          ("vt_labi", [128, 768], BF16))


def build_nc(debug=False):
    nc = bacc.Bacc("TRN2", target_bir_lowering=False, debug=False,
                   num_devices=NCORES)
    dt = {}
    for name in ("cond", "proc", "drug"):
        dt["emb_" + name] = nc.dram_tensor("emb_" + name, [VOCAB[name], D],
                                           BF16, kind="ExternalInput")
        dt["idx_" + name] = nc.dram_tensor("idx_" + name, [128, VIDX // 16],
                                           I16, kind="ExternalInput")
    for _, tag, _, nvc in OHSPEC:
        dt[tag] = nc.dram_tensor(tag, [128, NCHUNK * nvc * TOKC], F8,
                                 kind="ExternalInput")
    for name, shape, dty in CONSTS:
        dt[name] = nc.dram_tensor(name, shape, dty, kind="ExternalInput")
    out_logits = nc.dram_tensor("logits", [BL, OUT], F32, kind="ExternalOutput")
    if debug:
        dt["dbg_labT"] = nc.dram_tensor("dbg_labT", [128, GC], F32,
                                        kind="ExternalOutput")
        dt["dbg_h1"] = nc.dram_tensor("dbg_h1", [128, 160], F32,
                                      kind="ExternalOutput")
        dt["dbg_hfin"] = nc.dram_tensor("dbg_hfin", [128, 160], F32,
                                        kind="ExternalOutput")

    with tile.TileContext(nc) as tc:
        _program(nc, tc, dt, out_logits, debug)
    nc.compile()
    return nc


def _program(nc, tc, dt, out_logits, debug=False):
    import contextlib
    ctx = contextlib.ExitStack()
    with ctx:
        cpool = ctx.enter_context(tc.tile_pool(name="const", bufs=1))
        vgpool = ctx.enter_context(tc.tile_pool(name="vgather", bufs=1))
        ohpool = ctx.enter_context(tc.tile_pool(name="oh", bufs=2))
        spool = ctx.enter_context(tc.tile_pool(name="work", bufs=2))
        xgpool = ctx.enter_context(tc.tile_pool(name="xg", bufs=2))
        hpool = ctx.enter_context(tc.tile_pool(name="h", bufs=2))
        ppool = ctx.enter_context(tc.tile_pool(name="psum", bufs=2,
                                               space="PSUM"))

        cb = {}
        for name, shape, dty in CONSTS:
            t = cpool.tile(shape, dty, tag=name, name=name)
            nc.sync.dma_start(t[:], dt[name].ap())
            cb[name] = t
        idx = {}
        for name in ("cond", "proc", "drug"):
            t = cpool.tile([128, VIDX // 16], I16, tag="idx_" + name,
                           name="idx_" + name)
            nc.sync.dma_start(t[:], dt["idx_" + name].ap())
            idx[name] = t

        # ---- visit-level features eT[k] = [128 d, 32 bv] (fp16)
        eTb = {}
        for name in ("cond", "proc", "drug"):
            vG = vgpool.tile([128, VIDX], BF16, tag="vG_" + name, name="vG")
            nc.gpsimd.dma_gather(
                vG[:].rearrange("p (r e) -> p r e", e=D),
                dt["emb_" + name].ap(), idx[name][:], VIDX, VIDX, D)
            eTp = ppool.tile([128, NBV], F32, tag="pxg", name="eTp")
            for r in range(6):
                nc.tensor.matmul(eTp[:],
                                 vG[:].rearrange("p (r e) -> p r e", e=D)[:, r, :],
                                 cb["S4"][:], start=(r == 0), stop=(r == 5))
            et = cpool.tile([128, NBV], BF16, tag="eT_" + name, name="eT")
            nc.scalar.copy(et[:], eTp[:])
            eTb[name] = et

        # ---- xgc for monitor keys 0-2: [128, 3, 96] (gi, k*32+bv), bias folded
        xgc = cpool.tile([128, 3, 96], F32, tag="xgc", name="xgc")
        for k, name in enumerate(("cond", "proc", "drug")):
            pk = ppool.tile([128, 96], F32, tag="pxg", name="pk")
            for gi in range(3):
                nc.tensor.matmul(pk[:, gi * 32:(gi + 1) * 32],
                                 cb["mwihT012"][:, (k * 3 + gi) * 128:
                                                (k * 3 + gi + 1) * 128],
                                 eTb[name][:], start=True, stop=True)
            for gi in range(3):
                nc.scalar.activation(xgc[:, gi, k * 32:(k + 1) * 32],
                                     pk[:, gi * 32:(gi + 1) * 32], AF.Identity,
                                     bias=cb["mb012c"][:, k * 3 + gi:
                                                       k * 3 + gi + 1])

        # ---- visit xg for keys 5,6 (weight/age), rank-1 + const
        vxg = cpool.tile([128, 3, 7, NBV], F32, tag="vxg", name="vxg")
        for k in (5, 6):
            p56 = ppool.tile([128, 96], F32, tag="pxg", name="p56")
            for gi in range(3):
                nc.tensor.matmul(p56[:, gi * 32:(gi + 1) * 32],
                                 cb["vxg56u"][0:1, ((k - 5) * 3 + gi) * 128:
                                              ((k - 5) * 3 + gi + 1) * 128],
                                 cb["wa"][0:1, (k - 5) * 32:(k - 4) * 32],
                                 start=True, stop=True)
            for gi in range(3):
                nc.scalar.activation(vxg[:, gi, k, :],
                                     p56[:, gi * 32:(gi + 1) * 32], AF.Identity,
                                     bias=cb["vb56c"][:, (k - 5) * 3 + gi:
                                                      (k - 5) * 3 + gi + 1])

        # ---- monitor chunks + GRU chain
        h = hpool.tile([128, 160], BF16, tag="h", name="h0")
        nc.vector.memset(h[:], 0.0)

        for c in range(NCHUNK):
            ohs = {}
            for _, tag, _, nvc in OHSPEC:
                t = ohpool.tile([128, nvc, TOKC], F8, tag=tag, name="ohT")
                nc.sync.dma_start(
                    t[:], dt[tag].ap()
                    .rearrange("p (c x) -> p c x", c=NCHUNK)[:, c, :]
                    .rearrange("p (v t) -> p v t", v=nvc))
                ohs[tag] = t
            prod3 = spool.tile([128, TOKC], BF16, tag="prod3", name="prod3")
            prod4 = spool.tile([128, TOKC], BF16, tag="prod4", name="prod4")
            for blk in range(6):
                sl = slice(blk * 512, (blk + 1) * 512)
                for kk, vtag, votag, itag, iotag, nivc, prodX in (
                        (3, "vt_lab", "oh_lab", "vt_labi", "oh_labi", 6, prod3),
                        (4, "vt_inj", "oh_inj", "vt_inji", "oh_inji", 4, prod4)):
                    valP = ppool.tile([128, 512], F32, tag="valP", name="valP",
                                      bufs=3)
                    for vc in range(2):
                        nc.tensor.matmul(valP[:],
                                         cb[vtag][:, vc * 128:(vc + 1) * 128],
                                         ohs[votag][:, vc, sl],
                                         start=(vc == 0), stop=(vc == 1))
                    valS = spool.tile([128, 512], BF16, tag=f"valS{kk}",
                                      name="valS")
                    if blk % 2 == 0:
                        nc.scalar.copy(valS[:], valP[:])
                    else:
                        nc.vector.tensor_copy(valS[:], valP[:])
                    itemP = ppool.tile([128, 512], F32, tag="valP",
                                       name="itemP", bufs=3)
                    for vc in range(nivc):
                        nc.tensor.matmul(itemP[:],
                                         cb[itag][:, vc * 128:(vc + 1) * 128],
                                         ohs[iotag][:, vc, sl],
                                         start=(vc == 0), stop=(vc == nivc - 1))
                    nc.vector.tensor_tensor(prodX[:, sl], itemP[:], valS[:],
                                            op=mybir.AluOpType.mult)
            xg34c = xgpool.tile([128, 3, TCH, 64], F32, tag="xg34c",
                                name="xg34c")
            for k, prodX in ((3, prod3), (4, prod4)):
                red = spool.tile([128, GC], F32, tag=f"red{k}", name="red")
                nc.vector.tensor_reduce(
                    red[:], prodX[:].rearrange("p (g l) -> p g l", l=L),
                    axis=mybir.AxisListType.X, op=mybir.AluOpType.add)
                redb = spool.tile([128, GC], BF16, tag=f"redb{k}", name="redb")
                nc.scalar.copy(redb[:], red[:])
                pxg = ppool.tile([128, 384], F32, tag="pxg", name="pxg")
                for gi in range(3):
                    nc.tensor.matmul(pxg[:, gi * 128:(gi + 1) * 128],
                                     cb["mwihT34"][:, ((k - 3) * 3 + gi) * 128:
                                                   ((k - 3) * 3 + gi + 1) * 128],
                                     redb[:], start=True, stop=True)
                for gi in range(3):
                    nc.scalar.activation(
                        xg34c[:, gi, :, (k - 3) * 32:(k - 2) * 32],
                        pxg[:, gi * 128:(gi + 1) * 128]
                        .rearrange("p (m b) -> p m b", m=TCH), AF.Identity,
                        bias=cb["mb34c"][:, (k - 3) * 3 + gi:
                                         (k - 3) * 3 + gi + 1])
                if debug and c == 0 and k == 3:
                    nc.sync.dma_start(dt["dbg_labT"].ap(), red[:])

            for mi in range(TCH):
                przt = ppool.tile([128, 320], F32, tag="prz", name="przt",
                                  bufs=1)
                prz = przt[:].rearrange("p (g x) -> p g x", g=2)
                pn = ppool.tile([128, 160], F32, tag="pn", name="pn", bufs=1)
                nc.scalar.copy(prz[:, :, 0:96], xgc[:, 0:2, :])
                nc.scalar.copy(prz[:, :, 96:160], xg34c[:, 0:2, mi, :])
                for k in range(5):
                    hs = h[:, k * 32:(k + 1) * 32]
                    for gi in range(2):
                        nc.tensor.matmul(
                            prz[:, gi, k * 32:(k + 1) * 32],
                            cb["mwhhT"][:, (k * 3 + gi) * 128:
                                        (k * 3 + gi + 1) * 128],
                            hs, start=False, stop=True, skip_group_check=True)
                    nc.tensor.matmul(
                        pn[:, k * 32:(k + 1) * 32],
                        cb["mwhhT"][:, (k * 3 + 2) * 128:(k * 3 + 3) * 128],
                        hs, start=True, stop=True)
                r = spool.tile([128, 160], F32, tag="r", name="r")
                nc.scalar.activation(r[:], przt[:, 0:160], AF.Sigmoid)
                z = spool.tile([128, 160], BF16, tag="z", name="z")
                nc.scalar.activation(z[:], przt[:, 160:320], AF.Sigmoid)
                u = spool.tile([128, 160], F32, tag="u", name="u")
                nc.vector.tensor_tensor(u[:], pn[:], cb["mbhn_bc"][:],
                                        op=mybir.AluOpType.add)
                nc.vector.tensor_tensor(u[:], r[:], u[:],
                                        op=mybir.AluOpType.mult)
                npre = spool.tile([128, 160], F32, tag="npre", name="npre")
                nc.vector.tensor_tensor(npre[:, 0:96], u[:, 0:96],
                                        xgc[:, 2, :], op=mybir.AluOpType.add)
                nc.vector.tensor_tensor(npre[:, 96:160], u[:, 96:160],
                                        xg34c[:, 2, mi, :],
                                        op=mybir.AluOpType.add)
                nt = spool.tile([128, 160], BF16, tag="nt", name="nt")
                nc.scalar.activation(nt[:], npre[:], AF.Tanh)
                t3 = spool.tile([128, 160], BF16, tag="t3", name="t3")
                nc.vector.tensor_tensor(t3[:], h[:], nt[:],
                                        op=mybir.AluOpType.subtract)
                nc.vector.tensor_tensor(t3[:], t3[:], z[:],
                                        op=mybir.AluOpType.mult)
                h = hpool.tile([128, 160], BF16, tag="h", name="h")
                nc.vector.tensor_tensor(h[:], t3[:], nt[:],
                                        op=mybir.AluOpType.add)
                if debug and c == 0 and mi == 0:
                    hf = spool.tile([128, 160], F32, tag="hf", name="hf")
                    nc.vector.tensor_copy(hf[:], h[:])
                    nc.sync.dma_start(dt["dbg_h1"].ap(), hf[:])

        if debug:
            hf2 = spool.tile([128, 160], F32, tag="hf", name="hf2")
            nc.vector.tensor_copy(hf2[:], h[:])
            nc.sync.dma_start(dt["dbg_hfin"].ap(), hf2[:])

        # ---- visit xg for keys 0-4 from monitor hidden state
        for k in range(5):
            pk = ppool.tile([128, 96], F32, tag="pxg", name="pkv")
            for gi in range(3):
                nc.tensor.matmul(pk[:, gi * 32:(gi + 1) * 32],
                                 cb["vwihT04"][:, (k * 3 + gi) * 128:
                                               (k * 3 + gi + 1) * 128],
                                 h[:, k * 32:(k + 1) * 32],
                                 start=True, stop=True)
            for gi in range(3):
                nc.scalar.activation(vxg[:, gi, k, :],
                                     pk[:, gi * 32:(gi + 1) * 32], AF.Identity,
                                     bias=cb["vb04c"][:, k * 3 + gi:
                                                      k * 3 + gi + 1])

        # ---- visit GRU chain (7 keys, 16 steps, batch BL=2 per key)
        vxgv = vxg[:].rearrange("p g k (b v2) -> p g k b v2", b=BL)
        vh = hpool.tile([128, 14], BF16, tag="vh", name="vh0")
        nc.vector.memset(vh[:], 0.0)
        for v in range(V):
            pvrzt = ppool.tile([128, 28], F32, tag="prz", name="pvrzt",
                               bufs=1)
            pvrz = pvrzt[:].rearrange("p (g x) -> p g x", g=2)
            pvn = ppool.tile([128, 14], F32, tag="pn", name="pvn", bufs=1)
            nc.scalar.copy(pvrz[:, :, :], vxgv[:, 0:2, :, :, v])
            for k in range(7):
                hs = vh[:, k * 2:(k + 1) * 2]
                for gi in range(2):
                    nc.tensor.matmul(
                        pvrz[:, gi, k * 2:(k + 1) * 2],
                        cb["vwhhT"][:, (k * 3 + gi) * 128:
                                    (k * 3 + gi + 1) * 128],
                        hs, start=False, stop=True, skip_group_check=True)
                nc.tensor.matmul(
                    pvn[:, k * 2:(k + 1) * 2],
                    cb["vwhhT"][:, (k * 3 + 2) * 128:(k * 3 + 3) * 128],
                    hs, start=True, stop=True)
            vr = spool.tile([128, 14], F32, tag="vr", name="vr")
            nc.scalar.activation(vr[:], pvrzt[:, 0:14], AF.Sigmoid)
            vz = spool.tile([128, 14], BF16, tag="vz", name="vz")
            nc.scalar.activation(vz[:], pvrzt[:, 14:28], AF.Sigmoid)
            vu = spool.tile([128, 14], F32, tag="vu", name="vu")
            nc.vector.tensor_tensor(vu[:], pvn[:], cb["vbhn_bc"][:],
                                    op=mybir.AluOpType.add)
            nc.vector.tensor_tensor(vu[:], vr[:], vu[:],
                                    op=mybir.AluOpType.mult)
            nc.vector.tensor_tensor(vu[:], vu[:], vxgv[:, 2, :, :, v],
                                    op=mybir.AluOpType.add)
            vnt = spool.tile([128, 14], BF16, tag="vnt", name="vnt")
            nc.scalar.activation(vnt[:], vu[:], AF.Tanh)
            vt3 = spool.tile([128, 14], BF16, tag="vt3", name="vt3")
            nc.vector.tensor_tensor(vt3[:], vh[:], vnt[:],
                                    op=mybir.AluOpType.subtract)
            nc.vector.tensor_tensor(vt3[:], vt3[:], vz[:],
                                    op=mybir.AluOpType.mult)
            vh = hpool.tile([128, 14], BF16, tag="vh", name="vh")
            nc.vector.tensor_tensor(vh[:], vt3[:], vnt[:],
                                    op=mybir.AluOpType.add)

        # ---- FC head
        rlb = spool.tile([128, 14], BF16, tag="rlb", name="rlb")
        nc.scalar.activation(rlb[:], vh[:], AF.Relu)
        pfc = ppool.tile([BL, OUT], F32, tag="pn", name="pfc", bufs=1)
        for k in range(7):
            nc.tensor.matmul(pfc[:], rlb[:, k * 2:(k + 1) * 2],
                             cb["fcw"][:, k * OUT:(k + 1) * OUT],
                             start=(k == 0), stop=(k == 6))
        lg = spool.tile([BL, OUT], F32, tag="lg", name="lg")
        nc.vector.tensor_tensor(lg[:], pfc[:], cb["fcb2"][:],
                                op=mybir.AluOpType.add)
        nc.sync.dma_start(out_logits.ap(), lg[:])


# --------------------------------------------------------------------------
# entry point
# --------------------------------------------------------------------------

_NC_CACHE = None


def kernel(**inputs):
    global _NC_CACHE
    if _NC_CACHE is None:
        _NC_CACHE = build_nc()
    nc = _NC_CACHE
    shared = _prep_shared(inputs)
    in_maps = [_prep_core(inputs, shared, c) for c in range(NCORES)]
    res = run_bass_kernel_spmd(nc, in_maps, core_ids=list(range(NCORES)))
    return np.concatenate([res.results[c]["logits"] for c in range(NCORES)],
                          axis=0).astype(np.float32)


if __name__ == "__main__":
    import reference
    inputs = {k: np.asarray(v) for k, v in reference.setup_inputs().items()}
    out = kernel(**inputs)
    print("out", out.shape, out.dtype)


# revision 20
# speedup vs baseline: 1.0051x; 1.0051x over previous
"""Trainium2 Bass kernel for nn_CrossMed4 (CrossMed-style GRU-over-GRU model).

v4 strategy (8 NeuronCores, data-parallel over the patient batch B=16 -> 2/core):
- All four monitor embedding streams (lab/inj x item/value) are expanded on
  the PE from host-built fp8 one-hot matrices instead of SWDGE dma_gather
  (whose Q7 descriptor generation, ~8ns/row, was the machine bottleneck):
  stream[d, tok] = sum_vc tableT[vc, d]^T @ OH[vc, tok], accumulated over
  128-row vocab chunks in PSUM. fp8 one-hots are exact (0/1) and halve the
  HBM traffic; tables stay fp16.
- Token order t = g*24 + l (g = mi*32 + bv chunk-local group), so the pair
  product (DVE, fp16, one PSUM operand max) reduces over a contiguous
  innermost-24 axis straight into labT[d, g] -- no reduction matmuls, no
  transposes, no padding.
- Everything on the PE is 16-bit or fp8 (no fp32 hi/lo split, fast weight
  load); fp32 only in PSUM accumulation and the u-gate elementwise path.
- GRU state is fp16 end-to-end; r-gate stays fp32 for the u-path, z-gate is
  produced fp16 for the blend path. Gate biases are folded into ACT-engine
  Identity copies (per-partition bias); input projections accumulate onto
  ACT-preloaded PSUM. r/z and n gates live in SEPARATE PSUM banks (start=True
  marks the whole 2KB bank pending-zero, which would wipe the preload).
- Visit-level code features (cond/proc/drug) still use dma_gather (tiny:
  768 rows each) and reduce+transpose in one shot: six accumulating matmuls
  with the gathered rank as stationary and a 0/1 collapse matrix as moving.
"""
import numpy as np
import ml_dtypes

try:
    import concourse.bass as bass  # noqa: F401
except ImportError:
    import sys
    sys.path.insert(0, "/opt/trn_rl_repo")

import concourse.bacc as bacc
import concourse.bass as bass
import concourse.mybir as mybir
import concourse.tile as tile
from concourse.bass_utils import run_bass_kernel_spmd

F32 = mybir.dt.float32
F8 = mybir.dt.float8e4
BF16 = mybir.dt.float16
I16 = mybir.dt.int16
BF = np.float16
F8NP = ml_dtypes.float8_e4m3

B, V, M, L, D, OUT = 16, 16, 32, 24, 128, 193
VOCAB = {"cond": 5000, "proc": 2000, "drug": 600, "lab_item": 700,
         "lab_value": 200, "inj_item": 400, "inj_value": 200}
NCORES = 8
BL = B // NCORES            # 2 patients per core
NBV = BL * V                # 32 visit groups
TCH = 4                     # monitor steps per chunk
NCHUNK = M // TCH           # 8
GC = NBV * TCH              # 128 groups per chunk (= mi*32 + bv)
TOKC = GC * L               # 3072 tokens per chunk per stream
VIDX = NBV * L              # 768 visit-stream tokens (6 ranks)
AF = mybir.ActivationFunctionType

OHSPEC = (("lab_value", "oh_lab", "vt_lab", 2),
          ("inj_value", "oh_inj", "vt_inj", 2),
          ("inj_item", "oh_inji", "vt_inji", 4),
          ("lab_item", "oh_labi", "vt_labi", 6))


# --------------------------------------------------------------------------
# host-side packing
# --------------------------------------------------------------------------

def _wrap_idx(flat):
    # token i lives at [i % 16, i // 16]; the gather ucode's Q7 cores each
    # read their own 16-partition band, so replicate to all 8 bands.
    n = flat.shape[0]
    return np.tile(flat.reshape(n // 16, 16).T, (8, 1)).astype(np.int16)


def _packT(w_keys):  # [K, 3D, D] -> [128, K*3*128] fp16, col (k*3+gi)*128+d'
    k = w_keys.shape[0]
    out = np.zeros((128, k * 3 * 128), dtype=np.float32)
    for ki in range(k):
        for gi in range(3):
            out[:, (ki * 3 + gi) * 128:(ki * 3 + gi + 1) * 128] = \
                w_keys[ki, gi * 128:(gi + 1) * 128, :].T
    return out.astype(BF)


def _bias_cols(bih, bhh, keys):  # [128, len(keys)*3] f32, col k*3+gi
    cols = []
    for k in keys:
        for gi in range(3):
            b = bih[k][gi * D:(gi + 1) * D].copy()
            if gi < 2:
                b += bhh[k][gi * D:(gi + 1) * D]
            cols.append(b)
    return np.stack(cols, axis=1).astype(np.float32)


def _prep_shared(inputs):
    f = {k: np.asarray(v, dtype=np.float32) for k, v in inputs.items()
         if not k.startswith("tok_")}
    sh = {}
    for name in ("cond", "proc", "drug"):
        sh["emb_" + name] = f["emb_" + name].astype(BF)
    # tables packed for one-hot matmuls: vtp[p, c*128+d] = T[c*128+p, d]
    for name, _, vtag, nvc in OHSPEC:
        pad = np.zeros((nvc * 128, D), dtype=np.float32)
        pad[:VOCAB[name]] = f["emb_" + name]
        sh[vtag] = np.ascontiguousarray(
            pad.reshape(nvc, 128, D).transpose(1, 0, 2).reshape(128, nvc * 128)
        ).astype(BF)

    mwih, mwhh = f["mgru_wih"], f["mgru_whh"]
    mbih, mbhh = f["mgru_bih"], f["mgru_bhh"]
    vwih, vwhh = f["vgru_wih"], f["vgru_whh"]
    vbih, vbhh = f["vgru_bih"], f["vgru_bhh"]

    sh["mwhhT"] = _packT(mwhh)                   # [128, 1920]
    sh["mwihT012"] = _packT(mwih[0:3])           # [128, 1152]
    sh["mwihT34"] = _packT(mwih[3:5])            # [128, 768]
    sh["vwhhT"] = _packT(vwhh)                   # [128, 2688]
    sh["vwihT04"] = _packT(vwih[0:5])            # [128, 1920]
    sh["mb012c"] = _bias_cols(mbih, mbhh, [0, 1, 2])   # [128, 9]
    sh["mb34c"] = _bias_cols(mbih, mbhh, [3, 4])       # [128, 6]
    sh["vb04c"] = _bias_cols(vbih, vbhh, [0, 1, 2, 3, 4])  # [128, 15]
    sh["mbhn_bc"] = np.repeat(mbhh[:, 2 * D:].T, NBV, axis=1).astype(np.float32)
    sh["vbhn_bc"] = np.repeat(vbhh[:, 2 * D:].T, BL, axis=1).astype(np.float32)
    # keys 5,6 (weight/age): xg = u_k * wa[bv] + c_k  (rank-1)
    u_rows, c_cols = [], []
    for k in (5, 6):
        u_rows.append(vwih[k] @ f["info_w"][k - 5])
        cv = vwih[k] @ f["info_b"][k - 5] + vbih[k]
        cv[:2 * D] += vbhh[k][:2 * D]
        for gi in range(3):
            c_cols.append(cv[gi * D:(gi + 1) * D])
    sh["vxg56u"] = np.concatenate(u_rows)[None, :].astype(BF)   # [1, 768]
    sh["vb56c"] = np.stack(c_cols, axis=1).astype(np.float32)   # [128, 6]
    # collapse matrix for visit streams: S4[bv*4+cb, bv] = 1
    s4 = np.zeros((128, NBV), dtype=np.float32)
    for bv in range(NBV):
        s4[bv * 4:(bv + 1) * 4, bv] = 1.0
    sh["S4"] = s4.astype(BF)
    fcw = np.zeros((128, 7 * OUT), dtype=np.float32)
    for k in range(7):
        fcw[:, k * OUT:(k + 1) * OUT] = f["fc_w"][k * D:(k + 1) * D, :]
    sh["fcw"] = fcw.astype(BF)
    sh["fcb2"] = np.tile(f["fc_b"][None, :], (BL, 1)).astype(np.float32)
    return sh


def _mon_flat(tok):
    """tok [BL,V,M,L] int -> flat [NCHUNK*TOKC] with order
    flat[c*TOKC + (mi*NBV + b*V + v)*L + l] = tok[b, v, c*TCH+mi, l]."""
    t = np.asarray(tok).reshape(BL, V, NCHUNK, TCH, L)
    return np.ascontiguousarray(t.transpose(2, 3, 0, 1, 4)).reshape(-1)


def _prep_core(inputs, shared, core):
    b0 = core * BL
    m = dict(shared)
    # one-hot matrices [128, NCHUNK*nvc*TOKC] fp8 (0/1 exact)
    prow = np.arange(128)[:, None]
    for name, tag, _, nvc in OHSPEC:
        flat = _mon_flat(np.asarray(inputs["tok_" + name])[b0:b0 + BL])
        toks = flat.reshape(NCHUNK, TOKC)
        oh = np.zeros((128, NCHUNK, nvc, TOKC), dtype=F8NP)
        for c in range(NCHUNK):
            for vc in range(nvc):
                oh[:, c, vc, :] = (toks[c][None, :] == vc * 128 + prow)
        m[tag] = np.ascontiguousarray(oh).reshape(128, NCHUNK * nvc * TOKC)
    # visit-stream indices: flat[r*128 + (b*V+v)*4 + cb] = tok[b,v,cb*6+r]
    for name in ("cond", "proc", "drug"):
        t = np.asarray(inputs["tok_" + name])[b0:b0 + BL].reshape(BL, V, 4, 6)
        flat = np.ascontiguousarray(t.transpose(3, 0, 1, 2)).reshape(-1)
        m["idx_" + name] = _wrap_idx(flat)
    wa = np.zeros((1, 64), dtype=np.float32)
    wa[0, :NBV] = np.asarray(inputs["weight"], np.float32)[b0:b0 + BL].reshape(NBV)
    wa[0, NBV:] = np.asarray(inputs["age"], np.float32)[b0:b0 + BL].reshape(NBV)
    m["wa"] = wa.astype(BF)
    return m


# --------------------------------------------------------------------------
# device program
# --------------------------------------------------------------------------

CONSTS = (("mwhhT", [128, 1920], BF16), ("mwihT012", [128, 1152], BF16),
          ("mwihT34", [128, 768], BF16), ("vwhhT", [128, 2688], BF16),
          ("vwihT04", [128, 1920], BF16), ("mb012c", [128, 9], F32),
          ("mb34c", [128, 6], F32), ("vb04c", [128, 15], F32),
          ("mbhn_bc", [128, 160], F32), ("vbhn_bc", [128, 14], F32),
          ("vxg56u", [1, 768], BF16), ("vb56c", [128, 6], F32),
          ("S4", [128, NBV], BF16),
          ("fcw", [128, 7 * OUT], BF16), ("fcb2", [BL, OUT], F32),
          ("wa", [1, 64], BF16), ("vt_lab", [128, 256], BF16),
          ("vt_inj", [128, 256], BF16), ("vt_inji", [128, 512], BF16),
          ("vt_labi", [128, 768], BF16))


def build_nc(debug=False):
    nc = bacc.Bacc("TRN2", target_bir_lowering=False, debug=False,
                   num_devices=NCORES)
    dt = {}
    for name in ("cond", "proc", "drug"):
        dt["emb_" + name] = nc.dram_tensor("emb_" + name, [VOCAB[name], D],
                                           BF16, kind="ExternalInput")
        dt["idx_" + name] = nc.dram_tensor("idx_" + name, [128, VIDX // 16],
                                           I16, kind="ExternalInput")
    for _, tag, _, nvc in OHSPEC:
        dt[tag] = nc.dram_tensor(tag, [128, NCHUNK * nvc * TOKC], F8,
                                 kind="ExternalInput")
    for name, shape, dty in CONSTS:
        dt[name] = nc.dram_tensor(name, shape, dty, kind="ExternalInput")
    out_logits = nc.dram_tensor("logits", [BL, OUT], F32, kind="ExternalOutput")
    if debug:
        dt["dbg_labT"] = nc.dram_tensor("dbg_labT", [128, GC], F32,
                                        kind="ExternalOutput")
        dt["dbg_h1"] = nc.dram_tensor("dbg_h1", [128, 160], F32,
                                      kind="ExternalOutput")
        dt["dbg_hfin"] = nc.dram_tensor("dbg_hfin", [128, 160], F32,
                                        kind="ExternalOutput")

    with tile.TileContext(nc) as tc:
        _program(nc, tc, dt, out_logits, debug)
    nc.compile()
    return nc


def _program(nc, tc, dt, out_logits, debug=False):
    import contextlib
    ctx = contextlib.ExitStack()
    with ctx:
        cpool = ctx.enter_context(tc.tile_pool(name="const", bufs=1))
        vgpool = ctx.enter_context(tc.tile_pool(name="vgather", bufs=1))
        ohpool = ctx.enter_context(tc.tile_pool(name="oh", bufs=2))
        spool = ctx.enter_context(tc.tile_pool(name="work", bufs=2))
        xgpool = ctx.enter_context(tc.tile_pool(name="xg", bufs=2))
        hpool = ctx.enter_context(tc.tile_pool(name="h", bufs=2))
        ppool = ctx.enter_context(tc.tile_pool(name="psum", bufs=2,
                                               space="PSUM"))

        idx = {}
        for name in ("cond", "proc", "drug"):
            t = cpool.tile([128, VIDX // 16], I16, tag="idx_" + name,
                           name="idx_" + name)
            nc.sync.dma_start(t[:], dt["idx_" + name].ap())
            idx[name] = t

        def load_oh(c):
            ohs = {}
            for _, tag, _, nvc in OHSPEC:
                t = ohpool.tile([128, nvc, TOKC], F8, tag=tag, name="ohT")
                nc.sync.dma_start(
                    t[:], dt[tag].ap()
                    .rearrange("p (c x) -> p c x", c=NCHUNK)[:, c, :]
                    .rearrange("p (v t) -> p v t", v=nvc))
                ohs[tag] = t
            return ohs

        ohs0 = load_oh(0)

        cb = {}
        for name, shape, dty in CONSTS:
            t = cpool.tile(shape, dty, tag=name, name=name)
            nc.sync.dma_start(t[:], dt[name].ap())
            cb[name] = t

        # ---- visit-level features eT[k] = [128 d, 32 bv] (fp16)
        eTb = {}
        for name in ("cond", "proc", "drug"):
            vG = vgpool.tile([128, VIDX], BF16, tag="vG_" + name, name="vG")
            nc.gpsimd.dma_gather(
                vG[:].rearrange("p (r e) -> p r e", e=D),
                dt["emb_" + name].ap(), idx[name][:], VIDX, VIDX, D)
            eTp = ppool.tile([128, NBV], F32, tag="pxg", name="eTp")
            for r in range(6):
                nc.tensor.matmul(eTp[:],
                                 vG[:].rearrange("p (r e) -> p r e", e=D)[:, r, :],
                                 cb["S4"][:], start=(r == 0), stop=(r == 5))
            et = cpool.tile([128, NBV], BF16, tag="eT_" + name, name="eT")
            nc.scalar.copy(et[:], eTp[:])
            eTb[name] = et

        # ---- xgc for monitor keys 0-2: [128, 3, 96] (gi, k*32+bv), bias folded
        xgc = cpool.tile([128, 3, 96], F32, tag="xgc", name="xgc")
        for k, name in enumerate(("cond", "proc", "drug")):
            pk = ppool.tile([128, 96], F32, tag="pxg", name="pk")
            for gi in range(3):
                nc.tensor.matmul(pk[:, gi * 32:(gi + 1) * 32],
                                 cb["mwihT012"][:, (k * 3 + gi) * 128:
                                                (k * 3 + gi + 1) * 128],
                                 eTb[name][:], start=True, stop=True)
            for gi in range(3):
                nc.scalar.activation(xgc[:, gi, k * 32:(k + 1) * 32],
                                     pk[:, gi * 32:(gi + 1) * 32], AF.Identity,
                                     bias=cb["mb012c"][:, k * 3 + gi:
                                                       k * 3 + gi + 1])

        # ---- visit xg for keys 5,6 (weight/age), rank-1 + const
        vxg = cpool.tile([128, 3, 7, NBV], F32, tag="vxg", name="vxg")
        for k in (5, 6):
            p56 = ppool.tile([128, 96], F32, tag="pxg", name="p56")
            for gi in range(3):
                nc.tensor.matmul(p56[:, gi * 32:(gi + 1) * 32],
                                 cb["vxg56u"][0:1, ((k - 5) * 3 + gi) * 128:
                                              ((k - 5) * 3 + gi + 1) * 128],
                                 cb["wa"][0:1, (k - 5) * 32:(k - 4) * 32],
                                 start=True, stop=True)
            for gi in range(3):
                nc.scalar.activation(vxg[:, gi, k, :],
                                     p56[:, gi * 32:(gi + 1) * 32], AF.Identity,
                                     bias=cb["vb56c"][:, (k - 5) * 3 + gi:
                                                      (k - 5) * 3 + gi + 1])

        # ---- monitor chunks + GRU chain
        h = hpool.tile([128, 160], BF16, tag="h", name="h0")
        nc.vector.memset(h[:], 0.0)

        for c in range(NCHUNK):
            ohs = ohs0 if c == 0 else load_oh(c)
            prod3 = spool.tile([128, TOKC], BF16, tag="prod3", name="prod3")
            prod4 = spool.tile([128, TOKC], BF16, tag="prod4", name="prod4")
            for blk in range(6):
                sl = slice(blk * 512, (blk + 1) * 512)
                for kk, vtag, votag, itag, iotag, nivc, prodX in (
                        (3, "vt_lab", "oh_lab", "vt_labi", "oh_labi", 6, prod3),
                        (4, "vt_inj", "oh_inj", "vt_inji", "oh_inji", 4, prod4)):
                    valP = ppool.tile([128, 512], F32, tag="valP", name="valP",
                                      bufs=3)
                    for vc in range(2):
                        nc.tensor.matmul(valP[:],
                                         cb[vtag][:, vc * 128:(vc + 1) * 128],
                                         ohs[votag][:, vc, sl],
                                         start=(vc == 0), stop=(vc == 1))
                    valS = spool.tile([128, 512], BF16, tag=f"valS{kk}",
                                      name="valS")
                    if blk % 2 == 0:
                        nc.scalar.copy(valS[:], valP[:])
                    else:
                        nc.vector.tensor_copy(valS[:], valP[:])
                    itemP = ppool.tile([128, 512], F32, tag="valP",
                                       name="itemP", bufs=3)
                    for vc in range(nivc):
                        nc.tensor.matmul(itemP[:],
                                         cb[itag][:, vc * 128:(vc + 1) * 128],
                                         ohs[iotag][:, vc, sl],
                                         start=(vc == 0), stop=(vc == nivc - 1))
                    nc.vector.tensor_tensor(prodX[:, sl], itemP[:], valS[:],
                                            op=mybir.AluOpType.mult)
            xgall = xgpool.tile([128, 3, TCH, 160], F32, tag="xgall",
                                name="xgall")
            nc.scalar.copy(
                xgall[:, :, :, 0:96],
                xgc[:].unsqueeze(2).broadcast_to((128, 3, TCH, 96)))
            for k, prodX in ((3, prod3), (4, prod4)):
                pv = prodX[:].rearrange("p (g l) -> p g l", l=L)
                t1 = spool.tile([128, GC, 12], BF16, tag=f"t1{k}", name="t1")
                nc.gpsimd.tensor_tensor(t1[:], pv[:, :, 0:12], pv[:, :, 12:24],
                                        op=mybir.AluOpType.add)
                t2 = spool.tile([128, GC, 6], BF16, tag=f"t2{k}", name="t2")
                nc.gpsimd.tensor_tensor(t2[:], t1[:, :, 0:6], t1[:, :, 6:12],
                                        op=mybir.AluOpType.add)
                t3r = spool.tile([128, GC, 3], BF16, tag=f"t3{k}", name="t3r")
                nc.gpsimd.tensor_tensor(t3r[:], t2[:, :, 0:3], t2[:, :, 3:6],
                                        op=mybir.AluOpType.add)
                t4 = spool.tile([128, GC], BF16, tag=f"t4{k}", name="t4")
                nc.gpsimd.tensor_tensor(t4[:], t3r[:, :, 0], t3r[:, :, 1],
                                        op=mybir.AluOpType.add)
                redb = spool.tile([128, GC], BF16, tag=f"redb{k}", name="redb")
                nc.gpsimd.tensor_tensor(redb[:], t4[:], t3r[:, :, 2],
                                        op=mybir.AluOpType.add)
                if debug and c == 0 and k == 3:
                    redf = spool.tile([128, GC], F32, tag="redf", name="redf")
                    nc.vector.tensor_copy(redf[:], redb[:])
                    nc.sync.dma_start(dt["dbg_labT"].ap(), redf[:])
                pxg = ppool.tile([128, 384], F32, tag="pxg", name="pxg")
                for gi in range(3):
                    nc.tensor.matmul(pxg[:, gi * 128:(gi + 1) * 128],
                                     cb["mwihT34"][:, ((k - 3) * 3 + gi) * 128:
                                                   ((k - 3) * 3 + gi + 1) * 128],
                                     redb[:], start=True, stop=True)
                for gi in range(3):
                    nc.scalar.activation(
                        xgall[:, gi, :, 96 + (k - 3) * 32:96 + (k - 2) * 32],
                        pxg[:, gi * 128:(gi + 1) * 128]
                        .rearrange("p (m b) -> p m b", m=TCH), AF.Identity,
                        bias=cb["mb34c"][:, (k - 3) * 3 + gi:
                                         (k - 3) * 3 + gi + 1])

            for mi in range(TCH):
                przt = ppool.tile([128, 320], F32, tag="prz", name="przt",
                                  bufs=1)
                prz = przt[:].rearrange("p (g x) -> p g x", g=2)
                pn = ppool.tile([128, 160], F32, tag="pn", name="pn", bufs=1)
                nc.scalar.copy(prz[:, :, :], xgall[:, 0:2, mi, :])
                nc.scalar.copy(pn[:], cb["mbhn_bc"][:])
                for k in range(5):
                    hs = h[:, k * 32:(k + 1) * 32]
                    for gi in range(2):
                        nc.tensor.matmul(
                            prz[:, gi, k * 32:(k + 1) * 32],
                            cb["mwhhT"][:, (k * 3 + gi) * 128:
                                        (k * 3 + gi + 1) * 128],
                            hs, start=False, stop=True, skip_group_check=True)
                    nc.tensor.matmul(
                        pn[:, k * 32:(k + 1) * 32],
                        cb["mwhhT"][:, (k * 3 + 2) * 128:(k * 3 + 3) * 128],
                        hs, start=False, stop=True, skip_group_check=True)
                r = spool.tile([128, 160], F32, tag="r", name="r")
                nc.scalar.activation(r[:], przt[:, 0:160], AF.Sigmoid)
                z = spool.tile([128, 160], BF16, tag="z", name="z")
                nc.scalar.activation(z[:], przt[:, 160:320], AF.Sigmoid)
                u = spool.tile([128, 160], F32, tag="u", name="u")
                nc.vector.tensor_tensor(u[:], r[:], pn[:],
                                        op=mybir.AluOpType.mult)
                npre = spool.tile([128, 160], F32, tag="npre", name="npre")
                nc.vector.tensor_tensor(npre[:], u[:], xgall[:, 2, mi, :],
                                        op=mybir.AluOpType.add)
                nt = spool.tile([128, 160], BF16, tag="nt", name="nt")
                nc.scalar.activation(nt[:], npre[:], AF.Tanh)
                t3 = spool.tile([128, 160], BF16, tag="t3", name="t3")
                nc.vector.tensor_tensor(t3[:], h[:], nt[:],
                                        op=mybir.AluOpType.subtract)
                nc.vector.tensor_tensor(t3[:], t3[:], z[:],
                                        op=mybir.AluOpType.mult)
                h = hpool.tile([128, 160], BF16, tag="h", name="h")
                nc.vector.tensor_tensor(h[:], t3[:], nt[:],
                                        op=mybir.AluOpType.add)
                if debug and c == 0 and mi == 0:
                    hf = spool.tile([128, 160], F32, tag="hf", name="hf")
                    nc.vector.tensor_copy(hf[:], h[:])
                    nc.sync.dma_start(dt["dbg_h1"].ap(), hf[:])

        if debug:
            hf2 = spool.tile([128, 160], F32, tag="hf", name="hf2")
            nc.vector.tensor_copy(hf2[:], h[:])
            nc.sync.dma_start(dt["dbg_hfin"].ap(), hf2[:])

        # ---- visit xg for keys 0-4 from monitor hidden state
        for k in range(5):
            pk = ppool.tile([128, 96], F32, tag="pxg", name="pkv")
            for gi in range(3):
                nc.tensor.matmul(pk[:, gi * 32:(gi + 1) * 32],
                                 cb["vwihT04"][:, (k * 3 + gi) * 128:
                                               (k * 3 + gi + 1) * 128],
                                 h[:, k * 32:(k + 1) * 32],
                                 start=True, stop=True)
            for gi in range(3):
                nc.scalar.activation(vxg[:, gi, k, :],
                                     pk[:, gi * 32:(gi + 1) * 32], AF.Identity,
                                     bias=cb["vb04c"][:, k * 3 + gi:
                                                      k * 3 + gi + 1])

        # ---- visit GRU chain (7 keys, 16 steps, batch BL=2 per key)
        vxgv = vxg[:].rearrange("p g k (b v2) -> p g k b v2", b=BL)
        vh = hpool.tile([128, 14], BF16, tag="vh", name="vh0")
        nc.vector.memset(vh[:], 0.0)
        for v in range(V):
            pvrzt = ppool.tile([128, 28], F32, tag="prz", name="pvrzt",
                               bufs=1)
            pvrz = pvrzt[:].rearrange("p (g x) -> p g x", g=2)
            pvn = ppool.tile([128, 14], F32, tag="pn", name="pvn", bufs=1)
            nc.scalar.copy(pvrz[:, :, :], vxgv[:, 0:2, :, :, v])
            nc.scalar.copy(pvn[:], cb["vbhn_bc"][:])
            for k in range(7):
                hs = vh[:, k * 2:(k + 1) * 2]
                for gi in range(2):
                    nc.tensor.matmul(
                        pvrz[:, gi, k * 2:(k + 1) * 2],
                        cb["vwhhT"][:, (k * 3 + gi) * 128:
                                    (k * 3 + gi + 1) * 128],
                        hs, start=False, stop=True, skip_group_check=True)
                nc.tensor.matmul(
                    pvn[:, k * 2:(k + 1) * 2],
                    cb["vwhhT"][:, (k * 3 + 2) * 128:(k * 3 + 3) * 128],
                    hs, start=False, stop=True, skip_group_check=True)
            vr = spool.tile([128, 14], F32, tag="vr", name="vr")
            nc.scalar.activation(vr[:], pvrzt[:, 0:14], AF.Sigmoid)
            vz = spool.tile([128, 14], BF16, tag="vz", name="vz")
            nc.scalar.activation(vz[:], pvrzt[:, 14:28], AF.Sigmoid)
            vu = spool.tile([128, 14], F32, tag="vu", name="vu")
            nc.vector.tensor_tensor(vu[:], vr[:], pvn[:],
                                    op=mybir.AluOpType.mult)
            nc.vector.tensor_tensor(vu[:], vu[:], vxgv[:, 2, :, :, v],
                                    op=mybir.AluOpType.add)
            vnt = spool.tile([128, 14], BF16, tag="vnt", name="vnt")
            nc.scalar.activation(vnt[:], vu[:], AF.Tanh)
            vt3 = spool.tile([128, 14], BF16, tag="vt3", name="vt3")
            nc.vector.tensor_tensor(vt3[:], vh[:], vnt[:],
                                    op=mybir.AluOpType.subtract)
            nc.vector.tensor_tensor(vt3[:], vt3[:], vz[:],
                                    op=mybir.AluOpType.mult)
            vh = hpool.tile([128, 14], BF16, tag="vh", name="vh")
            nc.vector.tensor_tensor(vh[:], vt3[:], vnt[:],
                                    op=mybir.AluOpType.add)

        # ---- FC head
        rlb = spool.tile([128, 14], BF16, tag="rlb", name="rlb")
        nc.scalar.activation(rlb[:], vh[:], AF.Relu)
        pfc = ppool.tile([BL, OUT], F32, tag="pxg", name="pfc")
        for k in range(7):
            nc.tensor.matmul(pfc[:], rlb[:, k * 2:(k + 1) * 2],
                             cb["fcw"][:, k * OUT:(k + 1) * OUT],
                             start=(k == 0), stop=(k == 6))
        lg = spool.tile([BL, OUT], F32, tag="lg", name="lg")
        nc.vector.tensor_tensor(lg[:], pfc[:], cb["fcb2"][:],
                                op=mybir.AluOpType.add)
        nc.sync.dma_start(out_logits.ap(), lg[:])


# --------------------------------------------------------------------------
# entry point
# --------------------------------------------------------------------------

_NC_CACHE = None


def kernel(**inputs):
    global _NC_CACHE
    if _NC_CACHE is None:
        _NC_CACHE = build_nc()
    nc = _NC_CACHE
    shared = _prep_shared(inputs)
    in_maps = [_prep_core(inputs, shared, c) for c in range(NCORES)]
    res = run_bass_kernel_spmd(nc, in_maps, core_ids=list(range(NCORES)))
    return np.concatenate([res.results[c]["logits"] for c in range(NCORES)],
                          axis=0).astype(np.float32)


if __name__ == "__main__":
    import reference
    inputs = {k: np.asarray(v) for k, v in reference.setup_inputs().items()}
    out = kernel(**inputs)
    print("out", out.shape, out.dtype)
